# revision 1
# baseline (speedup 1.0000x reference)
"""Trainium2 Bass kernel for nn_HEMoETorch_43722767073393 (moe_routing).

Reference computation:
    h        = embed[x]                                  (N=4096, D=1024)
    h_fast   = relu(h @ fast_w1.T)
    scores   = exp(-max(||h-mu||^2, 0) / (2*sigma^2)) * charge     (N, 64)
    top_idx  = top_k(scores.mean(0), 8); top_w = scores[:, top_idx]
    slow_out = sum_k top_w[:,k] * (h @ expert_w[top_idx[k]].T)
    out      = (h_fast + 0.3 * slow_out) @ fast_w2.T     (N, 50257)

Numerical structure exploited: with D=1024, ||h - mu||^2 is ~1280 +- 60 for
every (token, expert) pair, so exp(-sq/8) < 1e-55 underflows to exactly 0.0
in fp32 for ALL pairs.  Hence top_w == 0 and slow_out == 0 *exactly* in the
fp32 reference, and the output is exactly relu(embed[x] @ W1^T) @ W2^T.
We verify this on the host (same fp32 underflow semantics); if it ever did
not hold we fall back to adding the host-computed slow term.

Device strategy (8 NeuronCores, no collectives):
  - replicate tokens: every core holds h^T for all 4096 tokens (bf16)
  - phase A (replicated): hf^T = relu(W1 @ h^T)        8.6 GF/core
  - phase C (vocab-sharded): each core computes logits[:, shard] where the
    50257-wide vocab dim of fast_w2 is split 8 ways     52.9 GF/core
  - all matmuls bf16 with fp32 PSUM accumulation
"""

import numpy as np
import ml_dtypes

import concourse.bass as bass  # noqa: F401  (bass must import before bacc)
import concourse.mybir as mybir
import concourse.tile as tile
from concourse import bacc
from concourse.bass_utils import run_bass_kernel_spmd

BF16 = ml_dtypes.bfloat16

N_CORES = 8
B, S = 4, 1024
N = B * S            # 4096 tokens
D = 1024
V = 50257
VS = 6283            # ceil(V / 8); padded total = 50264
V_PAD = VS * N_CORES
JT = D // 128        # 8 contraction tiles
NBLK = N // 128      # 32 token blocks (phase C output partition blocks)
NFREE = N // 512     # 8 token free-dim chunks (phase A)
VCH = 512            # vocab free-dim chunk
NVCH = (VS + VCH - 1) // VCH   # 13 chunks: 12x512 + 139
VB = 50              # ceil(VS/128): 128-wide vocab blocks (padded to 6400)
SIGMA = 2.0
FAST_RATIO = 0.7
TOP_K = 8

_prog_cache: dict = {}


def build_program(with_fast: bool = True, N=N, D=D, VS=VS, num_devices=N_CORES,
                  reps: int = 1):
    """Build the per-core SPMD program.

    with_fast=True : device computes hf^T = relu(W1 @ h^T), then logits.
    with_fast=False: input "ht" already holds h_merged^T; only the logits
                     matmul runs (host fallback path).
    """
    JT = D // 128
    NFREE = N // 512
    VB = (VS + 127) // 128
    nc = bacc.Bacc("TRN2", target_bir_lowering=False, debug=False,
                   num_devices=num_devices)
    bf = mybir.dt.bfloat16
    f32 = mybir.dt.float32

    ht_d = nc.dram_tensor("ht", [D, N], bf, kind="ExternalInput").ap()
    if with_fast:
        w1t_d = nc.dram_tensor("w1t", [D, D], bf, kind="ExternalInput").ap()
    w2p_d = nc.dram_tensor("w2p", [VB * 128, D], bf, kind="ExternalInput").ap()
    out_d = nc.dram_tensor("out", [VB * 128, N], f32, kind="ExternalOutput").ap()

    G = 4  # psum banks per accumulation group (8 total, 2-deep pipeline)
    NG = NFREE // G

    with tile.TileContext(nc) as tc:
        with (
            tc.tile_pool(name="persist", bufs=1) as persist,
            tc.tile_pool(name="w2s", bufs=4) as w2s,
            tc.tile_pool(name="ostage", bufs=8) as ostage,
            tc.tile_pool(name="psum", bufs=8, space="PSUM") as psum,
        ):
          with (tc.For_i(0, reps, 1) if reps > 1
                else __import__("contextlib").nullcontext()):
            # resident h^T tiles: partition = d (j-block), free = tokens
            ht = []
            for j in range(JT):
                t = persist.tile([128, N], bf, tag=f"ht{j}")
                nc.sync.dma_start(t[:], ht_d[j * 128:(j + 1) * 128, :])
                ht.append(t)

            if with_fast:
                w1 = []
                for j in range(JT):
                    t = persist.tile([128, D], bf, tag=f"w1_{j}")
                    nc.sync.dma_start(t[:], w1t_d[j * 128:(j + 1) * 128, :])
                    w1.append(t)
                # phase A: hf^T[i-block, :] = relu(sum_j W1^T[j,:]^T h^T[j,:])
                # j-outer within each token group so the stationary operand
                # (w1 column block) is reused across G streamed matmuls.
                hf = [persist.tile([128, N], bf, tag=f"hf{i}", name=f"hf{i}")
                      for i in range(JT)]
                for i in range(JT):
                    for g in range(NG):
                        pss = [psum.tile([128, 512], f32, tag="ps",
                                         name=f"psA{i}_{g}_{n}")
                               for n in range(G)]
                        for j in range(JT):
                            for n in range(G):
                                nt = g * G + n
                                nc.tensor.matmul(
                                    pss[n][:],
                                    w1[j][:, i * 128:(i + 1) * 128],
                                    ht[j][:, nt * 512:(nt + 1) * 512],
                                    start=(j == 0), stop=(j == JT - 1),
                                )
                        for n in range(G):
                            nt = g * G + n
                            nc.scalar.activation(
                                hf[i][:, nt * 512:(nt + 1) * 512], pss[n][:],
                                mybir.ActivationFunctionType.Relu,
                            )
            else:
                hf = ht

            # phase C (transposed): out^T[vb-block, tokens] accumulating over
            # d.  w2 block is the stationary operand, amortized over G
            # token-chunk streams; host pre-tiled w2p so each vb block is one
            # contiguous [128, JT*128] DMA.
            for vb in range(VB):
                w2c = w2s.tile([128, D], bf, tag="w2c")
                nc.sync.dma_start(w2c[:], w2p_d[vb * 128:(vb + 1) * 128, :])
                for g in range(NG):
                    pss = [psum.tile([128, 512], f32, tag="ps",
                                     name=f"psC{vb}_{g}_{n}")
                           for n in range(G)]
                    for j in range(JT):
                        for n in range(G):
                            nt = g * G + n
                            nc.tensor.matmul(
                                pss[n][:],
                                w2c[:, j * 128:(j + 1) * 128],
                                hf[j][:, nt * 512:(nt + 1) * 512],
                                start=(j == 0), stop=(j == JT - 1),
                            )
                    for n in range(G):
                        nt = g * G + n
                        ot = ostage.tile([128, 512], f32, tag="ot")
                        nc.vector.tensor_copy(ot[:], pss[n][:])
                        nc.sync.dma_start(
                            out_d[vb * 128:(vb + 1) * 128,
                                  nt * 512:(nt + 1) * 512],
                            ot[:],
                        )

    nc.compile()
    return nc


def _routing_host(x, embed, expert_mu, expert_charge):
    """fp32 host replica of the routing math (same underflow semantics as
    the jax fp32 reference).  Returns (top_idx, top_w)."""
    h = embed[x.reshape(-1)].astype(np.float32)                    # (N, D)
    sq = (
        np.sum(h * h, axis=1, keepdims=True)
        + np.sum(expert_mu * expert_mu, axis=1)[None, :]
        - 2.0 * (h @ expert_mu.T)
    ).astype(np.float32)
    kern = np.exp(-np.maximum(sq, 0.0) / np.float32(2.0 * SIGMA ** 2),
                  dtype=np.float32)
    scores = kern * expert_charge[None, :].astype(np.float32)
    mean = scores.mean(axis=0, dtype=np.float32)
    # jax.lax.top_k: descending by value, ties broken by lower index
    top_idx = np.lexsort((np.arange(mean.shape[0]), -mean))[:TOP_K]
    return top_idx, scores[:, top_idx], h


def prepare_inputs(x, embed, fast_w1, fast_w2, expert_mu, expert_w,
                   expert_charge):
    """Host-side shard prep. Returns (with_fast, in_maps)."""
    x = np.asarray(x).astype(np.int64).reshape(-1)
    embed = np.asarray(embed, dtype=np.float32)
    fast_w1 = np.asarray(fast_w1, dtype=np.float32)
    fast_w2 = np.asarray(fast_w2, dtype=np.float32)
    expert_mu = np.asarray(expert_mu, dtype=np.float32)
    expert_charge = np.asarray(expert_charge, dtype=np.float32)

    top_idx, top_w, h = _routing_host(x, embed, expert_mu, expert_charge)

    if not np.any(top_w):
        # expected path: slow branch is exactly zero
        with_fast = True
        ht = np.ascontiguousarray(h.T).astype(BF16)                # (D, N)
        w1t = np.ascontiguousarray(fast_w1.T).astype(BF16)         # (D, D)
    else:  # pragma: no cover - degenerate-input safety net
        with_fast = False
        expert_w = np.asarray(expert_w, dtype=np.float32)
        h_fast = np.maximum(h @ fast_w1.T, 0.0)
        slow = np.zeros_like(h_fast)
        for k in range(TOP_K):
            slow += top_w[:, k:k + 1] * (h @ expert_w[top_idx[k]].T)
        hm = h_fast + np.float32(1.0 - FAST_RATIO) * slow
        ht = np.ascontiguousarray(hm.T).astype(BF16)
        w1t = None

    w2tb = fast_w2.T.astype(BF16)                                  # (D, V)
    VB = 50
    w2t_full = np.zeros((D, VB * 128 * N_CORES), dtype=BF16)
    w2t_full[:, :V] = w2tb

    in_maps = []
    for c in range(N_CORES):
        # pre-tile the shard so each 128-wide vocab block is one contiguous
        # [128, D] DMA: w2p[vb*128+p, j*128+vcol] = w2T[j*128+p, vb*128+vcol]
        sh = w2t_full[:, :V][:, c * VS:(c + 1) * VS]
        shp = np.zeros((D, VB * 128), dtype=BF16)
        shp[:, :sh.shape[1]] = sh
        # [j, p, vb, vcol] -> [vb, p, j, vcol] -> (VB*128, D)
        w2p = np.ascontiguousarray(
            shp.reshape(JT, 128, VB, 128).transpose(2, 1, 0, 3)
        ).reshape(VB * 128, D)
        m = {"ht": ht, "w2p": w2p}
        if with_fast:
            m["w1t"] = w1t
        in_maps.append(m)
    return with_fast, in_maps


def kernel(**inputs) -> np.ndarray:
    with_fast, in_maps = prepare_inputs(**inputs)
    key = with_fast
    if key not in _prog_cache:
        _prog_cache[key] = build_program(with_fast)
    nc = _prog_cache[key]
    res = run_bass_kernel_spmd(nc, in_maps, core_ids=list(range(N_CORES)))
    # per-core output is transposed logits (VB*128, N); trim pad, stack, T
    shards = [res.results[c]["out"][:VS] for c in range(N_CORES)]
    full_t = np.concatenate(shards, axis=0)[:V]      # (V, N)
    return np.ascontiguousarray(full_t.T)

